# revision 56
# baseline (speedup 1.0000x reference)
"""DiffJPEG forward (16x3x512x512, quality=80) on 8 TRN2 NeuronCores.

Strategy: pure data-parallel over batch (2 images/core). Per core the JPEG
pipeline runs on-chip as 4 PE matmul stages (form-b / form-a alternation —
form-b stages use the data as the stationary operand, transposing for free),
everything in fp16 except the fp32 PSUM accumulators and quant scratch:

  S1 (form-b):  G1 = X^T A^T            vertical DCT (1 cyc/row at N=128)
  S2 (form-a):  F^T = sum_c' L[c,c'] G1_c'   horizontal DCT + fused 255*W_ycc
                                        color mix (N=512)
  quant      :  Q = round(F*(1/q))*q    fp32 magic-number round on DVE/ACT/
                                        GPSIMD, Q stored fp16
  S3 (form-b):  G3 = Q^T-chain = tq M per block
  S4 (form-a):  R = sum_c V[chan,c] M^T G3_c + 128/255 (fused inverse color
                                        mix, rank-1 DC-row bias)
  clip       :  out = clamp(R, 0, 1)    DVE, fp32 out

Precision: tolerance is 2e-2 L2-rel; the all-fp16 pipeline (11-bit operands,
fp32 accumulation, fp32 quant arithmetic) measures 8.9e-3 in exact numpy
emulation — 2.2x margin. Level shifts / color biases collapse into
DC-coefficient corrections (dca pattern on the forward, a rank-1 DC-row bias
on G3_y for the inverse, exact because beta*m0 == 128/255 by construction);
quality-dependent quant tables arrive as a tiny per-core [128,103] input pack
(the reference's qfull split over flattened (b,c)<16 means the luma/chroma
choice varies per core; global slice index = 6*core + local_slice).

Scheduling: PSUM tiles are 2-bank [128,1024]; elementwise ops cover 1024 cols
to halve per-op overheads. PSUM-reading work is split over the only two
engines with a PSUM port (DVE: quant multiply + output clips; ACT: S1/S3
psum->sbuf copies); the SBUF-only quant ops (magic add, magic subtract into
fp16 — exact since rounded values are integers <= 2048) alternate over
ACT/GPSIMD/DVE per QCONF (note: walrus rejects scalar_tensor_tensor on Pool,
so the dequant is sub + all-fp16 2x multiply on DVE). Junk matmuls at t=0
hold the PE p-state ramp through the input-load head; S3/S4 interleave per
column-pair so output DMA streams instead of flushing at the tail.
"""

import numpy as np

import concourse.bass as bass
import concourse.mybir as mybir
import concourse.tile as tile
from concourse import bacc
from concourse.bass_utils import run_bass_kernel_spmd

N_CORES = 8
BS = 16
IMGS_PER_CORE = BS // N_CORES          # 2
SLICES = IMGS_PER_CORE * 3             # 6
MAGIC = np.float32(1.5 * 2.0**23)      # fp32 round-to-nearest-even at ulp=1

F32 = mybir.dt.float32
F16 = mybir.dt.float16
COPY = mybir.ActivationFunctionType.Copy
IDENT = mybir.ActivationFunctionType.Identity
RELU = mybir.ActivationFunctionType.Relu

_LUM = np.array([[16,11,10,16,24,40,51,61],[12,12,14,19,26,58,60,55],[14,13,16,24,40,57,69,56],[14,17,22,29,51,87,80,62],[18,22,37,56,68,109,103,77],[24,35,55,64,81,104,113,92],[49,64,78,87,103,121,120,101],[72,92,95,98,112,100,103,99]], np.float32)
_CHROM = np.array([[17,18,24,47,99,99,99,99],[18,21,26,66,99,99,99,99],[24,26,56,99,99,99,99,99],[47,66,99,99,99,99,99,99],[99,99,99,99,99,99,99,99],[99,99,99,99,99,99,99,99],[99,99,99,99,99,99,99,99],[99,99,99,99,99,99,99,99]], np.float32)
_WYCC = np.array([[0.299, 0.587, 0.114], [-0.1687, -0.3313, 0.5], [0.5, -0.4187, -0.0813]], np.float32)
# inverse color terms: out_chan <- sum of coef * rec_channel (y=0, cb=1, cr=2)
_S4TERMS = [
    [(0, 1.0), (2, 1.402)],                       # r
    [(0, 1.0), (1, -0.34414), (2, -0.71414)],     # g
    [(0, 1.0), (1, 1.772)],                       # b
]

# Engine assignment for the quant chain, indexed per quant tile (cycled).
# magic: A=ACT, P=GPSIMD, V=DVE; deq: sttP/sttV = one scalar_tensor_tensor on
# GPSIMD/DVE, subP/subA/subV = magic-subtract on that engine + fp16 2x
# multiply on DVE. clip: V=DVE tensor_scalar, A=ACT double-relu.
# sched: fwd2 = both forwards then both inverses; mix = interleave images.
QCONF = {"magic": ["A", "P"], "deq": ["subV", "subP"], "clip": ["V"], "sched": "fwd2",
         "warm": 12, "dca": "P"}


def _dct_mat():
    k = np.arange(8)[:, None]
    n = np.arange(8)[None, :]
    norm = np.where(k == 0, np.sqrt(1.0 / 8.0), np.sqrt(2.0 / 8.0))
    return (norm * np.cos(np.pi / 8.0 * (n + 0.5) * k)).astype(np.float32)


def _qtables(quality):
    q = max(1, min(100, int(quality)))
    scale = 5000.0 / q if q < 50 else 200.0 - 2.0 * q
    tbs = np.stack([_LUM, _CHROM]) * np.float32(scale)
    return np.clip((tbs + 50.0) / 100.0, 1.0, 255.0).astype(np.float32)


def _host_constants():
    M = _dct_mat()
    BD = np.kron(np.eye(16, dtype=np.float32), M)       # kron(I16, M)
    BDT = np.ascontiguousarray(BD.T)                    # kron(I16, M^T)

    s13w = np.concatenate([BDT, BD], axis=1).astype(np.float16)  # [128,256]

    s2w = np.zeros((128, 9 * 128), np.float16)          # [p, 9n]: BDT*255*W
    for c in range(3):
        for cp in range(3):
            s2w[:, 128 * (3 * c + cp) : 128 * (3 * c + cp) + 128] = (
                BDT * np.float32(255.0 * _WYCC[c, cp])).astype(np.float16)

    s4w = np.zeros((128, 7 * 128), np.float16)          # [p, 7n]: BD*coef/255
    s4idx = {}
    wi = 0
    for chan in range(3):
        for (csrc, coef) in _S4TERMS[chan]:
            s4idx[(chan, csrc)] = wi
            s4w[:, 128 * wi : 128 * wi + 128] = (
                BD * np.float32(coef / 255.0)).astype(np.float16)
            wi += 1

    m128 = np.arange(128)
    # +128/255 output bias, folded into the y-channel S3-out copy: adding
    # beta at DCT-row-0 partitions of G3_y contributes beta*m0 per pixel
    # through every channel's (chan,0) S4 weight, where m0 is that weight's
    # DC-row entry. Choose beta so beta*m0 == 128/255 exactly.
    m0 = float(np.float32(s4w[0, 128 * s4idx[(0, 0)]]))
    beta = np.float32(np.float64(128.0 / 255.0) / m0)
    s3b = (np.float32(beta) * (m128 % 8 == 0)).astype(np.float32)[:, None]  # [128,1]
    return dict(s13w=s13w, s2w=s2w, s4w=s4w, s4idx=s4idx, s3b=s3b)


def _quant_inputs(quality, core, s3b):
    """Per-core quant-pattern pack [128, 103]: columns are rq [6x8], qq [6x8],
    dca [6], s3b [1].

    Quant runs on F^T laid out [v (partition), u (free)]:
    pattern value at (p, j) = qt[u=j, v=p%8]."""
    qt = _qtables(quality)
    pack = np.zeros((128, 2 * SLICES * 8 + SLICES + 1), np.float32)
    p = np.arange(128)
    for i in range(SLICES):
        g = 6 * core + i                      # global flattened (b,c) slice
        tab = qt[0] if g < BS else qt[1]
        # [128,8]: [p, j] = tab[j, p%8]
        pack[:, 8 * i : 8 * i + 8] = (1.0 / tab.astype(np.float64))[:, p % 8].T.astype(np.float32)
        pack[:, 8 * (SLICES + i) : 8 * (SLICES + i) + 8] = tab[:, p % 8].T
        # -1024 * (1/q[0,0]): the Y-channel DC level-shift applied post-rq-mult,
        # nonzero only on v%8==0 partitions (add of 0 elsewhere is a no-op)
        pack[p % 8 == 0, 16 * SLICES + i] = np.float32(
            -1024.0 * float(pack[0, 8 * i]))
    pack[:, 17 * SLICES] = s3b[:, 0]
    return pack


def _trace():
    hc = _host_constants()
    nc = bacc.Bacc("TRN2", target_bir_lowering=False, debug=False)

    NQ = 2 * SLICES * 8 + SLICES + 1
    img_d = nc.dram_tensor("img", [SLICES, 512, 512], F16, kind="ExternalInput").ap()
    qp_d = nc.dram_tensor("qpack", [128, NQ], F32, kind="ExternalInput").ap()
    s13w_d = nc.dram_tensor("s13w", [128, 256], F16, kind="ExternalInput").ap()
    s2w_d = nc.dram_tensor("s2w", [128, 9 * 128], F16, kind="ExternalInput").ap()
    s4w_d = nc.dram_tensor("s4w", [128, 7 * 128], F16, kind="ExternalInput").ap()
    # fp16 wire format for the output (host upcasts to f32): halves the
    # output DMA so it fits under the inverse-phase PE time with no tail.
    # Layout [im*2+sp, 128, (chan, slab, col)] matches the staging tiles so
    # each slab-pair's 3 channels ship as ONE contiguous DMA; the host
    # untangles it when unsharding.
    out_d = nc.dram_tensor("out", [2 * IMGS_PER_CORE, 128, 3072], F16, kind="ExternalOutput").ap()

    s4idx = hc["s4idx"]

    with tile.TileContext(nc) as tc:
        with (
            tc.tile_pool(name="wts", bufs=1) as wp,
            tc.tile_pool(name="img", bufs=2) as imp,
            tc.tile_pool(name="g1", bufs=2) as g1p,
            tc.tile_pool(name="qq", bufs=2) as qp,
            tc.tile_pool(name="g3", bufs=2) as g3p,
            tc.tile_pool(name="ost", bufs=3) as op,
            tc.tile_pool(name="scr", bufs=4) as sp,
            tc.tile_pool(name="psA", bufs=2, space="PSUM") as psAp,
            tc.tile_pool(name="psB", bufs=2, space="PSUM") as psBp,
        ):
            # img tile free layout: (half mtp, slab s, col c) so each plane
            # arrives as two 256-col DMAs and S1 can start on the first half.
            def load_plane(t, sl):
                for mtp in range(2):
                    nc.sync.dma_start(
                        t[:, 1024 * mtp : 1024 * mtp + 1024]
                        .rearrange("p (s c) -> p s c", s=4),
                        img_d[sl, :, 256 * mtp : 256 * mtp + 256]
                        .rearrange("(s p) c -> p s c", p=128),
                    )

            def xslice(t, w, mt):
                """[128,128] stationary slice: slab w, cols 128mt..128mt+128."""
                base = 1024 * (mt // 2) + 256 * w + 128 * (mt % 2)
                return t[:, base : base + 128]

            s13w = wp.tile([128, 256], F16, tag="s13w")
            nc.sync.dma_start(s13w[:], s13w_d)
            s1w = s13w[:, 0:128]
            s3w = s13w[:, 128:256]

            # PE warmup: junk matmuls on a memset tile (no DMA dependency)
            # keep the tensor engine continuously busy through the preamble
            # and input-load head, so the p-state ramp completes before the
            # real pipeline starts.
            wgarb = wp.tile([128, 256], F16, tag="wgarb")
            nc.vector.memset(wgarb[:], 0.0)

            def s1fill(n):
                """Filler matmuls: absorb input-DMA pacing gaps and hold the
                PE ramp. Fresh pool tiles so psA rotation is not pinned."""
                done = 0
                while done < n:
                    k = min(4, n - done)
                    wu = psAp.tile([128, 1024], F32, tag="psA")
                    for r in range(k):
                        nc.tensor.matmul(
                            wu[:, 256 * r : 256 * r + 256],
                            wgarb[:, 0:128], wgarb[:],
                            start=True, stop=True,
                        )
                    done += k

            s1fill(QCONF.get("warm", 16))

            early_imgs = []
            for _c in range(3):
                _t = imp.tile([128, 2048], F16, tag=f"x{_c}")
                load_plane(_t, _c)
                early_imgs.append(_t)
            qpk = wp.tile([128, NQ], F32, tag="qpk")
            nc.sync.dma_start(qpk[:], qp_d)
            O_RQ, O_QQ, O_DCA, O_S3B = 0, SLICES * 8, 16 * SLICES, 17 * SLICES
            s2w = wp.tile([128, 9 * 128], F16, tag="s2w")
            nc.sync.dma_start(s2w[:], s2w_d)
            s4w = wp.tile([128, 7 * 128], F16, tag="s4w")
            nc.sync.dma_start(s4w[:], s4w_d)
            # fp16 copy of the dequant patterns for the all-fp16 dequant mult
            qq16 = wp.tile([128, SLICES * 8], F16, tag="qq16")
            nc.scalar.activation(qq16[:], qpk[:, O_QQ : O_QQ + SLICES * 8], COPY)

            state = {}

            def s_load(im):
                if im == 0:
                    state[("x", 0)] = early_imgs
                    return
                xt = []
                for c in range(3):
                    t = imp.tile([128, 2048], F16, tag=f"x{c}")
                    load_plane(t, 3 * im + c)
                    xt.append(t)
                state[("x", im)] = xt

            def s1(im, chans=(0, 1, 2)):
                """Vertical DCT: per (c, mt-pair) one [128,1024] psum of 8
                fp16 matmuls, then one ACT copy into g1 (fp16)."""
                xt = state[("x", im)]
                g1 = state.setdefault(("g1", im), [None, None, None])
                for c in chans:
                    g_t = g1p.tile([128, 2048], F16, tag=f"g1_{c}")
                    g1[c] = g_t
                    for mtp in range(2):
                        ps = psAp.tile([128, 1024], F32, tag="psA")
                        for k in range(2):
                            mt = 2 * mtp + k
                            for w in range(4):
                                nc.tensor.matmul(
                                    ps[:, 512 * k + 128 * w : 512 * k + 128 * w + 128],
                                    xslice(xt[c], w, mt),
                                    s1w,
                                    start=True, stop=True,
                                )
                        nc.scalar.activation(
                            g_t[:, 1024 * mtp : 1024 * mtp + 1024], ps[:], COPY)
                    s1fill(QCONF.get("s1fill", 0))

            def s2q(im, chans=(0, 1, 2)):
                """Horizontal DCT + color mix + quantization.

                Per (c, s-pair): [128,1024] psum from 2x3 fp16 matmuls, then
                quant chain: DVE mult-by-1/q (+DC adjust), fp32 magic-round
                add (ACT/GPSIMD alternating), GPSIMD dequant-scale to fp16."""
                g1 = state[("g1", im)]
                qt_ = state.setdefault(("q", im), [None, None, None])
                for c in chans:
                    q = qp.tile([128, 2048], F16, tag=f"q_{c}")
                    qt_[c] = q
                    sl = 3 * im + c
                    rqv = qpk[:, O_RQ + 8 * sl : O_RQ + 8 * sl + 8].rearrange("p (o j) -> p o j", o=1).broadcast_to((128, 128, 8))
                    qqv16 = qq16[:, 8 * sl : 8 * sl + 8].rearrange("p (o j) -> p o j", o=1).broadcast_to((128, 128, 8))
                    for sp_ in range(2):
                        ps = psBp.tile([128, 1024], F32, tag="psB")
                        for k in range(2):
                            s = 2 * sp_ + k
                            for cp in range(3):
                                nc.tensor.matmul(
                                    ps[:, 512 * k : 512 * k + 512],
                                    s2w[:, 128 * (3 * c + cp) : 128 * (3 * c + cp) + 128],
                                    g1[cp][:, 512 * s : 512 * s + 512],
                                    start=(cp == 0), stop=(cp == 2),
                                )
                        tb = sp.tile([128, 1024], F32, tag="tq")
                        nc.vector.tensor_tensor(
                            tb[:].rearrange("p (a j) -> p a j", j=8),
                            ps[:].rearrange("p (a j) -> p a j", j=8),
                            rqv, op=mybir.AluOpType.mult,
                        )
                        if c == 0:
                            deng = nc.gpsimd if QCONF.get("dca") == "P" else nc.vector
                            deng.tensor_scalar_add(
                                tb[:, 0:1024:8], tb[:, 0:1024:8],
                                qpk[:, O_DCA + sl : O_DCA + sl + 1],
                            )
                        ti_ = 6 * im + 2 * c + sp_
                        qs = q[:, 1024 * sp_ : 1024 * sp_ + 1024].rearrange("p (a j) -> p a j", j=8)
                        # magic round-to-int add
                        meng = QCONF["magic"][ti_ % len(QCONF["magic"])]
                        if meng == "A":
                            nc.scalar.activation(tb[:], tb[:], COPY, bias=float(MAGIC))
                        elif meng == "P":
                            nc.gpsimd.tensor_scalar_add(tb[:], tb[:], float(MAGIC))
                        else:
                            nc.vector.tensor_scalar_add(tb[:], tb[:], float(MAGIC))
                        deq = QCONF["deq"][ti_ % len(QCONF["deq"])]
                        if deq.startswith("stt"):
                            # single scalar_tensor_tensor dequant
                            eng = nc.gpsimd if deq == "sttP" else nc.vector
                            eng.scalar_tensor_tensor(
                                qs, tb[:].rearrange("p (a j) -> p a j", j=8),
                                float(MAGIC), qqv16,
                                op0=mybir.AluOpType.subtract,
                                op1=mybir.AluOpType.mult,
                            )
                        else:
                            # two-op: subtract magic into fp16 (exact), then
                            # all-fp16 2x multiply on DVE
                            tr = sp.tile([128, 1024], F16, tag="tr")
                            seng = deq[3]
                            if seng == "P":
                                nc.gpsimd.tensor_scalar_sub(tr[:], tb[:], float(MAGIC))
                            elif seng == "A":
                                nc.scalar.activation(tr[:], tb[:], COPY, bias=-float(MAGIC))
                            else:
                                nc.vector.tensor_scalar_sub(tr[:], tb[:], float(MAGIC))
                            nc.vector.tensor_tensor(
                                qs, tr[:].rearrange("p (a j) -> p a j", j=8),
                                qqv16, op=mybir.AluOpType.mult,
                            )

            def s3(im, mtps=(0, 1), chans=(0, 1, 2)):
                """Horizontal inverse DCT (fp16 form-b). Needs all of q[c]."""
                qt_ = state[("q", im)]
                g3 = state.setdefault(("g3", im), [None, None, None])
                for mtp in mtps:
                    for c in chans:
                        if g3[c] is None or mtp == 0:
                            if mtp == 0:
                                g3_t = g3p.tile([128, 2048], F16, tag=f"g3_{c}")
                                g3[c] = g3_t
                        ps = psAp.tile([128, 1024], F32, tag="psA")
                        for k in range(2):
                            mt = 2 * mtp + k
                            for c2 in range(4):
                                nc.tensor.matmul(
                                    ps[:, 512 * k + 128 * c2 : 512 * k + 128 * c2 + 128],
                                    qt_[c][:, 512 * c2 + 128 * mt : 512 * c2 + 128 * mt + 128],
                                    s3w,
                                    start=True, stop=True,
                                )
                        if c == 0:
                            nc.scalar.activation(
                                g3[c][:, 1024 * mtp : 1024 * mtp + 1024], ps[:], IDENT,
                                bias=qpk[:, O_S3B : O_S3B + 1],
                            )
                        else:
                            nc.scalar.activation(
                                g3[c][:, 1024 * mtp : 1024 * mtp + 1024], ps[:], COPY
                            )

            def s4(im, sps=(0, 1)):
                """Vertical inverse DCT + inverse color mix + clip + store.

                Slab-pair sp only needs g3[:, 1024sp:+1024] = s3(im, mtp=sp).
                One DVE clip into the staging tile, one [128,1024] DMA."""
                g3 = state[("g3", im)]
                for sp_ in sps:
                    # staging tile covers all 3 channels of this slab-pair;
                    # each channel ships as its own [128,1024] DMA as soon as
                    # its clip lands (earliest overlap, 2-dim APs)
                    ot_t = op.tile([128, 3072], F16, tag="ot")
                    for chan in range(3):
                        terms = _S4TERMS[chan]
                        ps = psBp.tile([128, 1024], F32, tag="psB")
                        for k in range(2):
                            s = 2 * sp_ + k
                            for ti, (csrc, _) in enumerate(terms):
                                wi = s4idx[(chan, csrc)]
                                nc.tensor.matmul(
                                    ps[:, 512 * k : 512 * k + 512],
                                    s4w[:, 128 * wi : 128 * wi + 128],
                                    g3[csrc][:, 512 * s : 512 * s + 512],
                                    start=(ti == 0), stop=(ti == len(terms) - 1),
                                )
                        ots = ot_t[:, 1024 * chan : 1024 * chan + 1024]
                        ci = 6 * im + 3 * sp_ + chan
                        if QCONF["clip"][ci % len(QCONF["clip"])] == "A":
                            # clip via two Relus on ACT: y = relu(1-relu(1-x))
                            rt = sp.tile([128, 1024], F32, tag="rl")
                            nc.scalar.activation(rt[:], ps[:], RELU, bias=1.0, scale=-1.0)
                            nc.scalar.activation(ots, rt[:], RELU, bias=1.0, scale=-1.0)
                        else:
                            nc.vector.tensor_scalar(
                                ots, ps[:], 0.0, 1.0,
                                op0=mybir.AluOpType.max, op1=mybir.AluOpType.min,
                            )
                        oqs = QCONF.get("oq", ["S"])
                        oq = oqs[ci % len(oqs)]
                        oeng = {"S": nc.sync, "A": nc.scalar, "P": nc.gpsimd}[oq]
                        oeng.dma_start(
                            out_d[2 * im + sp_, :, 1024 * chan : 1024 * chan + 1024],
                            ots,
                        )

            # software-pipelined schedule; S3/S4 interleave per column-pair
            # so outputs stream early instead of flushing at the tail.
            s_load(0)
            s_load(1)
            if QCONF["sched"] == "fwd2":
                # both images' forward stages first (wide window for the
                # quant chains to hide behind PE work), then the inverses
                s1(0)
                s2q(0)
                s1(1)
                s2q(1)
                for im in (0, 1):
                    s3(im, mtps=(0,))
                    s4(im, sps=(0,))
                    s3(im, mtps=(1,))
                    s4(im, sps=(1,))
            elif QCONF["sched"] == "s1first":
                # both S1s first: PE never blocks in-order on the s2w DMA,
                # and input-paced S1 work covers the whole load head
                s1(0)
                s1(1)
                s2q(0)
                s2q(1)
                for im in (0, 1):
                    s3(im, mtps=(0,))
                    s4(im, sps=(0,))
                    s3(im, mtps=(1,))
                    s4(im, sps=(1,))
            elif QCONF["sched"] == "hyb":
                # image-1 forward partially interleaved so image-0 outputs
                # start early and output DMA spreads across the whole run
                s1(0)
                s2q(0)
                s1(1)
                s2q(1, chans=(0,))
                s3(0, mtps=(0,))
                s4(0, sps=(0,))
                s2q(1, chans=(1,))
                s3(0, mtps=(1,))
                s4(0, sps=(1,))
                s2q(1, chans=(2,))
                s3(1, mtps=(0,))
                s4(1, sps=(0,))
                s3(1, mtps=(1,))
                s4(1, sps=(1,))
            else:
                s1(0)
                s2q(0)
                s1(1)
                s3(0, mtps=(0,))
                s4(0, sps=(0,))
                s2q(1, chans=(0, 1))
                s3(0, mtps=(1,))
                s4(0, sps=(1,))
                s2q(1, chans=(2,))
                s3(1, mtps=(0,))
                s4(1, sps=(0,))
                s3(1, mtps=(1,))
                s4(1, sps=(1,))
    nc.compile()
    return nc, hc


_COMPILED = None


def _get_compiled():
    global _COMPILED
    if _COMPILED is None:
        _COMPILED = _trace()
    return _COMPILED


def kernel(img, quality):
    img = np.asarray(img)
    quality = int(np.asarray(quality))
    nc, hc = _get_compiled()

    img16 = np.ascontiguousarray(img.astype(np.float16))
    in_maps = []
    for core in range(N_CORES):
        qpack = _quant_inputs(quality, core, hc["s3b"])
        shard = np.ascontiguousarray(
            img16[IMGS_PER_CORE * core : IMGS_PER_CORE * (core + 1)].reshape(SLICES, 512, 512)
        )
        in_maps.append({
            "img": shard, "qpack": qpack,
            "s13w": hc["s13w"], "s2w": hc["s2w"], "s4w": hc["s4w"],
        })

    res = run_bass_kernel_spmd(nc, in_maps, core_ids=list(range(N_CORES)))
    # wire layout per core: [im*2+sp, p, (chan, slab, col)] -> [2,3,512,512]
    out = np.stack([res.results[c]["out"] for c in range(N_CORES)])
    out = out.reshape(N_CORES, IMGS_PER_CORE, 2, 128, 3, 2, 512)
    out = out.transpose(0, 1, 4, 2, 5, 3, 6)   # [core, im, ch, sp, s, p, col]
    return np.ascontiguousarray(out).reshape(BS, 3, 512, 512).astype(np.float32)


if __name__ == "__main__":
    rng = np.random.default_rng(0)
    x = rng.random((BS, 3, 512, 512), dtype=np.float32)
    y = kernel(x, 80)
    print("kernel ran:", y.shape, y.dtype, float(y.min()), float(y.max()))


# revision 61
# speedup vs baseline: 1.0014x; 1.0014x over previous
"""DiffJPEG forward (16x3x512x512, quality=80) on 8 TRN2 NeuronCores.

Strategy: pure data-parallel over batch (2 images/core). Per core the JPEG
pipeline runs on-chip as 4 PE matmul stages (form-b / form-a alternation —
form-b stages use the data as the stationary operand, transposing for free),
everything in fp16 except the fp32 PSUM accumulators and quant scratch:

  S1 (form-b):  G1 = X^T A^T            vertical DCT (1 cyc/row at N=128)
  S2 (form-a):  F^T = sum_c' L[c,c'] G1_c'   horizontal DCT + fused 255*W_ycc
                                        color mix (N=512)
  quant      :  Q = round(F*(1/q))*q    fp32 magic-number round on DVE/ACT/
                                        GPSIMD, Q stored fp16
  S3 (form-b):  G3 = Q^T-chain = tq M per block
  S4 (form-a):  R = sum_c V[chan,c] M^T G3_c + 128/255 (fused inverse color
                                        mix, rank-1 DC-row bias)
  clip       :  out = clamp(R, 0, 1)    DVE, fp32 out

Precision: tolerance is 2e-2 L2-rel; the all-fp16 pipeline (11-bit operands,
fp32 accumulation, fp32 quant arithmetic) measures 8.9e-3 in exact numpy
emulation — 2.2x margin. Level shifts / color biases collapse into
DC-coefficient corrections (dca pattern on the forward, a rank-1 DC-row bias
on G3_y for the inverse, exact because beta*m0 == 128/255 by construction);
quality-dependent quant tables arrive as a tiny per-core [128,103] input pack
(the reference's qfull split over flattened (b,c)<16 means the luma/chroma
choice varies per core; global slice index = 6*core + local_slice).

Scheduling: PSUM tiles are 2-bank [128,1024]; elementwise ops cover 1024 cols
to halve per-op overheads. PSUM-reading work is split over the only two
engines with a PSUM port (DVE: quant multiply + output clips; ACT: S1/S3
psum->sbuf copies); the SBUF-only quant ops (magic add, magic subtract into
fp16 — exact since rounded values are integers <= 2048) alternate over
ACT/GPSIMD/DVE per QCONF (note: walrus rejects scalar_tensor_tensor on Pool,
so the dequant is sub + all-fp16 2x multiply on DVE). Junk matmuls at t=0
hold the PE p-state ramp through the input-load head; S3/S4 interleave per
column-pair so output DMA streams instead of flushing at the tail.
"""

import numpy as np

import concourse.bass as bass
import concourse.mybir as mybir
import concourse.tile as tile
from concourse import bacc
from concourse.bass_utils import run_bass_kernel_spmd

N_CORES = 8
BS = 16
IMGS_PER_CORE = BS // N_CORES          # 2
SLICES = IMGS_PER_CORE * 3             # 6
MAGIC = np.float32(1.5 * 2.0**23)      # fp32 round-to-nearest-even at ulp=1

F32 = mybir.dt.float32
F16 = mybir.dt.float16
COPY = mybir.ActivationFunctionType.Copy
IDENT = mybir.ActivationFunctionType.Identity
RELU = mybir.ActivationFunctionType.Relu

_LUM = np.array([[16,11,10,16,24,40,51,61],[12,12,14,19,26,58,60,55],[14,13,16,24,40,57,69,56],[14,17,22,29,51,87,80,62],[18,22,37,56,68,109,103,77],[24,35,55,64,81,104,113,92],[49,64,78,87,103,121,120,101],[72,92,95,98,112,100,103,99]], np.float32)
_CHROM = np.array([[17,18,24,47,99,99,99,99],[18,21,26,66,99,99,99,99],[24,26,56,99,99,99,99,99],[47,66,99,99,99,99,99,99],[99,99,99,99,99,99,99,99],[99,99,99,99,99,99,99,99],[99,99,99,99,99,99,99,99],[99,99,99,99,99,99,99,99]], np.float32)
_WYCC = np.array([[0.299, 0.587, 0.114], [-0.1687, -0.3313, 0.5], [0.5, -0.4187, -0.0813]], np.float32)
# inverse color terms: out_chan <- sum of coef * rec_channel (y=0, cb=1, cr=2)
_S4TERMS = [
    [(0, 1.0), (2, 1.402)],                       # r
    [(0, 1.0), (1, -0.34414), (2, -0.71414)],     # g
    [(0, 1.0), (1, 1.772)],                       # b
]

# Engine assignment for the quant chain, indexed per quant tile (cycled).
# magic: A=ACT, P=GPSIMD, V=DVE; deq: sttP/sttV = one scalar_tensor_tensor on
# GPSIMD/DVE, subP/subA/subV = magic-subtract on that engine + fp16 2x
# multiply on DVE. clip: V=DVE tensor_scalar, A=ACT double-relu.
# sched: fwd2 = both forwards then both inverses; mix = interleave images.
QCONF = {"magic": ["A", "P"], "deq": ["subV", "subP"], "clip": ["V"], "sched": "fwd2",
         "warm": 12, "dca": "P", "s4ord": (1, 0, 2)}


def _dct_mat():
    k = np.arange(8)[:, None]
    n = np.arange(8)[None, :]
    norm = np.where(k == 0, np.sqrt(1.0 / 8.0), np.sqrt(2.0 / 8.0))
    return (norm * np.cos(np.pi / 8.0 * (n + 0.5) * k)).astype(np.float32)


def _qtables(quality):
    q = max(1, min(100, int(quality)))
    scale = 5000.0 / q if q < 50 else 200.0 - 2.0 * q
    tbs = np.stack([_LUM, _CHROM]) * np.float32(scale)
    return np.clip((tbs + 50.0) / 100.0, 1.0, 255.0).astype(np.float32)


def _host_constants():
    M = _dct_mat()
    BD = np.kron(np.eye(16, dtype=np.float32), M)       # kron(I16, M)
    BDT = np.ascontiguousarray(BD.T)                    # kron(I16, M^T)

    s13w = np.concatenate([BDT, BD], axis=1).astype(np.float16)  # [128,256]

    s2w = np.zeros((128, 9 * 128), np.float16)          # [p, 9n]: BDT*255*W
    for c in range(3):
        for cp in range(3):
            s2w[:, 128 * (3 * c + cp) : 128 * (3 * c + cp) + 128] = (
                BDT * np.float32(255.0 * _WYCC[c, cp])).astype(np.float16)

    s4w = np.zeros((128, 7 * 128), np.float16)          # [p, 7n]: BD*coef/255
    s4idx = {}
    wi = 0
    for chan in range(3):
        for (csrc, coef) in _S4TERMS[chan]:
            s4idx[(chan, csrc)] = wi
            s4w[:, 128 * wi : 128 * wi + 128] = (
                BD * np.float32(coef / 255.0)).astype(np.float16)
            wi += 1

    m128 = np.arange(128)
    # +128/255 output bias, folded into the y-channel S3-out copy: adding
    # beta at DCT-row-0 partitions of G3_y contributes beta*m0 per pixel
    # through every channel's (chan,0) S4 weight, where m0 is that weight's
    # DC-row entry. Choose beta so beta*m0 == 128/255 exactly.
    m0 = float(np.float32(s4w[0, 128 * s4idx[(0, 0)]]))
    beta = np.float32(np.float64(128.0 / 255.0) / m0)
    s3b = (np.float32(beta) * (m128 % 8 == 0)).astype(np.float32)[:, None]  # [128,1]
    return dict(s13w=s13w, s2w=s2w, s4w=s4w, s4idx=s4idx, s3b=s3b)


def _quant_inputs(quality, core, s3b):
    """Per-core quant-pattern pack [128, 103]: columns are rq [6x8], qq [6x8],
    dca [6], s3b [1].

    Quant runs on F^T laid out [v (partition), u (free)]:
    pattern value at (p, j) = qt[u=j, v=p%8]."""
    qt = _qtables(quality)
    pack = np.zeros((128, 2 * SLICES * 8 + SLICES + 1), np.float32)
    p = np.arange(128)
    for i in range(SLICES):
        g = 6 * core + i                      # global flattened (b,c) slice
        tab = qt[0] if g < BS else qt[1]
        # [128,8]: [p, j] = tab[j, p%8]
        pack[:, 8 * i : 8 * i + 8] = (1.0 / tab.astype(np.float64))[:, p % 8].T.astype(np.float32)
        pack[:, 8 * (SLICES + i) : 8 * (SLICES + i) + 8] = tab[:, p % 8].T
        # -1024 * (1/q[0,0]): the Y-channel DC level-shift applied post-rq-mult,
        # nonzero only on v%8==0 partitions (add of 0 elsewhere is a no-op)
        pack[p % 8 == 0, 16 * SLICES + i] = np.float32(
            -1024.0 * float(pack[0, 8 * i]))
    pack[:, 17 * SLICES] = s3b[:, 0]
    return pack


def _trace():
    hc = _host_constants()
    nc = bacc.Bacc("TRN2", target_bir_lowering=False, debug=False)

    NQ = 2 * SLICES * 8 + SLICES + 1
    img_d = nc.dram_tensor("img", [SLICES, 512, 512], F16, kind="ExternalInput").ap()
    qp_d = nc.dram_tensor("qpack", [128, NQ], F32, kind="ExternalInput").ap()
    s13w_d = nc.dram_tensor("s13w", [128, 256], F16, kind="ExternalInput").ap()
    s2w_d = nc.dram_tensor("s2w", [128, 9 * 128], F16, kind="ExternalInput").ap()
    s4w_d = nc.dram_tensor("s4w", [128, 7 * 128], F16, kind="ExternalInput").ap()
    # fp16 wire format for the output (host upcasts to f32): halves the
    # output DMA so it fits under the inverse-phase PE time with no tail.
    # Layout [im*2+sp, 128, (chan, slab, col)] matches the staging tiles so
    # each slab-pair's 3 channels ship as ONE contiguous DMA; the host
    # untangles it when unsharding.
    out_d = nc.dram_tensor("out", [2 * IMGS_PER_CORE, 128, 3072], F16, kind="ExternalOutput").ap()

    s4idx = hc["s4idx"]

    with tile.TileContext(nc) as tc:
        with (
            tc.tile_pool(name="wts", bufs=1) as wp,
            tc.tile_pool(name="img", bufs=2) as imp,
            tc.tile_pool(name="g1", bufs=2) as g1p,
            tc.tile_pool(name="qq", bufs=2) as qp,
            tc.tile_pool(name="g3", bufs=2) as g3p,
            tc.tile_pool(name="ost", bufs=3) as op,
            tc.tile_pool(name="scr", bufs=4) as sp,
            tc.tile_pool(name="psA", bufs=2, space="PSUM") as psAp,
            tc.tile_pool(name="psB", bufs=2, space="PSUM") as psBp,
        ):
            # img tile free layout: (half mtp, slab s, col c) so each plane
            # arrives as two 256-col DMAs and S1 can start on the first half.
            def load_plane(t, sl):
                for mtp in range(2):
                    nc.sync.dma_start(
                        t[:, 1024 * mtp : 1024 * mtp + 1024]
                        .rearrange("p (s c) -> p s c", s=4),
                        img_d[sl, :, 256 * mtp : 256 * mtp + 256]
                        .rearrange("(s p) c -> p s c", p=128),
                    )

            def xslice(t, w, mt):
                """[128,128] stationary slice: slab w, cols 128mt..128mt+128."""
                base = 1024 * (mt // 2) + 256 * w + 128 * (mt % 2)
                return t[:, base : base + 128]

            s13w = wp.tile([128, 256], F16, tag="s13w")
            nc.sync.dma_start(s13w[:], s13w_d)
            s1w = s13w[:, 0:128]
            s3w = s13w[:, 128:256]

            # PE warmup: junk matmuls on a memset tile (no DMA dependency)
            # keep the tensor engine continuously busy through the preamble
            # and input-load head, so the p-state ramp completes before the
            # real pipeline starts.
            wgarb = wp.tile([128, 256], F16, tag="wgarb")
            nc.vector.memset(wgarb[:], 0.0)

            def s1fill(n):
                """Filler matmuls: absorb input-DMA pacing gaps and hold the
                PE ramp. Fresh pool tiles so psA rotation is not pinned."""
                done = 0
                while done < n:
                    k = min(4, n - done)
                    wu = psAp.tile([128, 1024], F32, tag="psA")
                    for r in range(k):
                        nc.tensor.matmul(
                            wu[:, 256 * r : 256 * r + 256],
                            wgarb[:, 0:128], wgarb[:],
                            start=True, stop=True,
                        )
                    done += k

            s1fill(QCONF.get("warm", 16))

            early_imgs = []
            for _c in range(3):
                _t = imp.tile([128, 2048], F16, tag=f"x{_c}")
                load_plane(_t, _c)
                early_imgs.append(_t)
            qpk = wp.tile([128, NQ], F32, tag="qpk")
            nc.sync.dma_start(qpk[:], qp_d)
            O_RQ, O_QQ, O_DCA, O_S3B = 0, SLICES * 8, 16 * SLICES, 17 * SLICES
            s2w = wp.tile([128, 9 * 128], F16, tag="s2w")
            # split upload: channel-0's three mix blocks land first so S2 can
            # start as soon as image 0 is resident
            nc.sync.dma_start(s2w[:, 0 : 3 * 128], s2w_d[:, 0 : 3 * 128])
            nc.sync.dma_start(s2w[:, 3 * 128 :], s2w_d[:, 3 * 128 :])
            s4w = wp.tile([128, 7 * 128], F16, tag="s4w")
            nc.sync.dma_start(s4w[:], s4w_d)
            # fp16 copy of the dequant patterns for the all-fp16 dequant mult
            qq16 = wp.tile([128, SLICES * 8], F16, tag="qq16")
            nc.scalar.activation(qq16[:], qpk[:, O_QQ : O_QQ + SLICES * 8], COPY)

            state = {}

            def s_load(im):
                if im == 0:
                    state[("x", 0)] = early_imgs
                    return
                xt = []
                for c in range(3):
                    t = imp.tile([128, 2048], F16, tag=f"x{c}")
                    load_plane(t, 3 * im + c)
                    xt.append(t)
                state[("x", im)] = xt

            def s1(im, chans=(0, 1, 2)):
                """Vertical DCT: per (c, mt-pair) one [128,1024] psum of 8
                fp16 matmuls, then one ACT copy into g1 (fp16)."""
                xt = state[("x", im)]
                g1 = state.setdefault(("g1", im), [None, None, None])
                for c in chans:
                    g_t = g1p.tile([128, 2048], F16, tag=f"g1_{c}")
                    g1[c] = g_t
                    for mtp in range(2):
                        ps = psAp.tile([128, 1024], F32, tag="psA")
                        for k in range(2):
                            mt = 2 * mtp + k
                            for w in range(4):
                                nc.tensor.matmul(
                                    ps[:, 512 * k + 128 * w : 512 * k + 128 * w + 128],
                                    xslice(xt[c], w, mt),
                                    s1w,
                                    start=True, stop=True,
                                )
                        s1c = QCONF.get("s1cp", ["A"])
                        if s1c[(6 * im + 2 * c + mtp) % len(s1c)] == "V":
                            nc.vector.tensor_scalar_add(
                                g_t[:, 1024 * mtp : 1024 * mtp + 1024], ps[:], 0.0)
                        else:
                            nc.scalar.activation(
                                g_t[:, 1024 * mtp : 1024 * mtp + 1024], ps[:], COPY)
                    s1fill(QCONF.get("s1fill", 0))

            def s2q(im, chans=(0, 1, 2)):
                """Horizontal DCT + color mix + quantization.

                Per (c, s-pair): [128,1024] psum from 2x3 fp16 matmuls, then
                quant chain: DVE mult-by-1/q (+DC adjust), fp32 magic-round
                add (ACT/GPSIMD alternating), GPSIMD dequant-scale to fp16."""
                g1 = state[("g1", im)]
                qt_ = state.setdefault(("q", im), [None, None, None])
                for c in chans:
                    q = qp.tile([128, 2048], F16, tag=f"q_{c}")
                    qt_[c] = q
                    sl = 3 * im + c
                    rqv = qpk[:, O_RQ + 8 * sl : O_RQ + 8 * sl + 8].rearrange("p (o j) -> p o j", o=1).broadcast_to((128, 128, 8))
                    qqv16 = qq16[:, 8 * sl : 8 * sl + 8].rearrange("p (o j) -> p o j", o=1).broadcast_to((128, 128, 8))
                    for sp_ in range(2):
                        ps = psBp.tile([128, 1024], F32, tag="psB")
                        for k in range(2):
                            s = 2 * sp_ + k
                            for cp in range(3):
                                nc.tensor.matmul(
                                    ps[:, 512 * k : 512 * k + 512],
                                    s2w[:, 128 * (3 * c + cp) : 128 * (3 * c + cp) + 128],
                                    g1[cp][:, 512 * s : 512 * s + 512],
                                    start=(cp == 0), stop=(cp == 2),
                                )
                        tb = sp.tile([128, 1024], F32, tag="tq")
                        nc.vector.tensor_tensor(
                            tb[:].rearrange("p (a j) -> p a j", j=8),
                            ps[:].rearrange("p (a j) -> p a j", j=8),
                            rqv, op=mybir.AluOpType.mult,
                        )
                        if c == 0:
                            deng = nc.gpsimd if QCONF.get("dca") == "P" else nc.vector
                            deng.tensor_scalar_add(
                                tb[:, 0:1024:8], tb[:, 0:1024:8],
                                qpk[:, O_DCA + sl : O_DCA + sl + 1],
                            )
                        ti_ = 6 * im + 2 * c + sp_
                        qs = q[:, 1024 * sp_ : 1024 * sp_ + 1024].rearrange("p (a j) -> p a j", j=8)
                        # magic round-to-int add
                        meng = QCONF["magic"][ti_ % len(QCONF["magic"])]
                        if meng == "A":
                            nc.scalar.activation(tb[:], tb[:], COPY, bias=float(MAGIC))
                        elif meng == "P":
                            nc.gpsimd.tensor_scalar_add(tb[:], tb[:], float(MAGIC))
                        else:
                            nc.vector.tensor_scalar_add(tb[:], tb[:], float(MAGIC))
                        deq = QCONF["deq"][ti_ % len(QCONF["deq"])]
                        if deq.startswith("stt"):
                            # single scalar_tensor_tensor dequant
                            eng = nc.gpsimd if deq == "sttP" else nc.vector
                            eng.scalar_tensor_tensor(
                                qs, tb[:].rearrange("p (a j) -> p a j", j=8),
                                float(MAGIC), qqv16,
                                op0=mybir.AluOpType.subtract,
                                op1=mybir.AluOpType.mult,
                            )
                        else:
                            # two-op: subtract magic into fp16 (exact), then
                            # all-fp16 2x multiply on DVE
                            tr = sp.tile([128, 1024], F16, tag="tr")
                            seng = deq[3]
                            if seng == "P":
                                nc.gpsimd.tensor_scalar_sub(tr[:], tb[:], float(MAGIC))
                            elif seng == "A":
                                nc.scalar.activation(tr[:], tb[:], COPY, bias=-float(MAGIC))
                            else:
                                nc.vector.tensor_scalar_sub(tr[:], tb[:], float(MAGIC))
                            t2s = QCONF.get("tt2", ["V"])
                            t2eng = nc.gpsimd if t2s[ti_ % len(t2s)] == "P" else nc.vector
                            t2eng.tensor_tensor(
                                qs, tr[:].rearrange("p (a j) -> p a j", j=8),
                                qqv16, op=mybir.AluOpType.mult,
                            )

            def s3(im, mtps=(0, 1), chans=(0, 1, 2)):
                """Horizontal inverse DCT (fp16 form-b). Needs all of q[c]."""
                qt_ = state[("q", im)]
                g3 = state.setdefault(("g3", im), [None, None, None])
                for mtp in mtps:
                    for c in chans:
                        if g3[c] is None or mtp == 0:
                            if mtp == 0:
                                g3_t = g3p.tile([128, 2048], F16, tag=f"g3_{c}")
                                g3[c] = g3_t
                        ps = psAp.tile([128, 1024], F32, tag="psA")
                        for k in range(2):
                            mt = 2 * mtp + k
                            for c2 in range(4):
                                nc.tensor.matmul(
                                    ps[:, 512 * k + 128 * c2 : 512 * k + 128 * c2 + 128],
                                    qt_[c][:, 512 * c2 + 128 * mt : 512 * c2 + 128 * mt + 128],
                                    s3w,
                                    start=True, stop=True,
                                )
                        if c == 0:
                            nc.scalar.activation(
                                g3[c][:, 1024 * mtp : 1024 * mtp + 1024], ps[:], IDENT,
                                bias=qpk[:, O_S3B : O_S3B + 1],
                            )
                        else:
                            nc.scalar.activation(
                                g3[c][:, 1024 * mtp : 1024 * mtp + 1024], ps[:], COPY
                            )

            def s4(im, sps=(0, 1)):
                """Vertical inverse DCT + inverse color mix + clip + store.

                Slab-pair sp only needs g3[:, 1024sp:+1024] = s3(im, mtp=sp).
                One DVE clip into the staging tile, one [128,1024] DMA."""
                g3 = state[("g3", im)]
                for sp_ in sps:
                    # staging tile covers all 3 channels of this slab-pair;
                    # each channel ships as its own [128,1024] DMA as soon as
                    # its clip lands (earliest overlap, 2-dim APs)
                    ot_t = op.tile([128, 3072], F16, tag="ot")
                    for chan in QCONF.get("s4ord", (0, 1, 2)):
                        terms = _S4TERMS[chan]
                        ps = psBp.tile([128, 1024], F32, tag="psB")
                        for k in range(2):
                            s = 2 * sp_ + k
                            for ti, (csrc, _) in enumerate(terms):
                                wi = s4idx[(chan, csrc)]
                                nc.tensor.matmul(
                                    ps[:, 512 * k : 512 * k + 512],
                                    s4w[:, 128 * wi : 128 * wi + 128],
                                    g3[csrc][:, 512 * s : 512 * s + 512],
                                    start=(ti == 0), stop=(ti == len(terms) - 1),
                                )
                        ots = ot_t[:, 1024 * chan : 1024 * chan + 1024]
                        ci = 6 * im + 3 * sp_ + chan
                        if QCONF["clip"][ci % len(QCONF["clip"])] == "A":
                            # clip via two Relus on ACT: y = relu(1-relu(1-x))
                            rt = sp.tile([128, 1024], F32, tag="rl")
                            nc.scalar.activation(rt[:], ps[:], RELU, bias=1.0, scale=-1.0)
                            nc.scalar.activation(ots, rt[:], RELU, bias=1.0, scale=-1.0)
                        else:
                            nc.vector.tensor_scalar(
                                ots, ps[:], 0.0, 1.0,
                                op0=mybir.AluOpType.max, op1=mybir.AluOpType.min,
                            )
                        oqs = QCONF.get("oq", ["S"])
                        oq = oqs[ci % len(oqs)]
                        oeng = {"S": nc.sync, "A": nc.scalar, "P": nc.gpsimd}[oq]
                        oeng.dma_start(
                            out_d[2 * im + sp_, :, 1024 * chan : 1024 * chan + 1024],
                            ots,
                        )

            # software-pipelined schedule; S3/S4 interleave per column-pair
            # so outputs stream early instead of flushing at the tail.
            s_load(0)
            s_load(1)
            if QCONF["sched"] == "fwd2":
                # both images' forward stages first (wide window for the
                # quant chains to hide behind PE work), then the inverses
                s1(0)
                s2q(0)
                s1(1)
                s2q(1)
                for im in (0, 1):
                    s3(im, mtps=(0,))
                    s4(im, sps=(0,))
                    s3(im, mtps=(1,))
                    s4(im, sps=(1,))
            elif QCONF["sched"] == "s1first":
                # both S1s first: PE never blocks in-order on the s2w DMA,
                # and input-paced S1 work covers the whole load head
                s1(0)
                s1(1)
                s2q(0)
                s2q(1)
                for im in (0, 1):
                    s3(im, mtps=(0,))
                    s4(im, sps=(0,))
                    s3(im, mtps=(1,))
                    s4(im, sps=(1,))
            elif QCONF["sched"] == "hyb":
                # image-1 forward partially interleaved so image-0 outputs
                # start early and output DMA spreads across the whole run
                s1(0)
                s2q(0)
                s1(1)
                s2q(1, chans=(0,))
                s3(0, mtps=(0,))
                s4(0, sps=(0,))
                s2q(1, chans=(1,))
                s3(0, mtps=(1,))
                s4(0, sps=(1,))
                s2q(1, chans=(2,))
                s3(1, mtps=(0,))
                s4(1, sps=(0,))
                s3(1, mtps=(1,))
                s4(1, sps=(1,))
            else:
                s1(0)
                s2q(0)
                s1(1)
                s3(0, mtps=(0,))
                s4(0, sps=(0,))
                s2q(1, chans=(0, 1))
                s3(0, mtps=(1,))
                s4(0, sps=(1,))
                s2q(1, chans=(2,))
                s3(1, mtps=(0,))
                s4(1, sps=(0,))
                s3(1, mtps=(1,))
                s4(1, sps=(1,))
    nc.compile()
    return nc, hc


_COMPILED = None


def _get_compiled():
    global _COMPILED
    if _COMPILED is None:
        _COMPILED = _trace()
    return _COMPILED


def kernel(img, quality):
    img = np.asarray(img)
    quality = int(np.asarray(quality))
    nc, hc = _get_compiled()

    img16 = np.ascontiguousarray(img.astype(np.float16))
    in_maps = []
    for core in range(N_CORES):
        qpack = _quant_inputs(quality, core, hc["s3b"])
        shard = np.ascontiguousarray(
            img16[IMGS_PER_CORE * core : IMGS_PER_CORE * (core + 1)].reshape(SLICES, 512, 512)
        )
        in_maps.append({
            "img": shard, "qpack": qpack,
            "s13w": hc["s13w"], "s2w": hc["s2w"], "s4w": hc["s4w"],
        })

    res = run_bass_kernel_spmd(nc, in_maps, core_ids=list(range(N_CORES)))
    # wire layout per core: [im*2+sp, p, (chan, slab, col)] -> [2,3,512,512]
    out = np.stack([res.results[c]["out"] for c in range(N_CORES)])
    out = out.reshape(N_CORES, IMGS_PER_CORE, 2, 128, 3, 2, 512)
    out = out.transpose(0, 1, 4, 2, 5, 3, 6)   # [core, im, ch, sp, s, p, col]
    return np.ascontiguousarray(out).reshape(BS, 3, 512, 512).astype(np.float32)


if __name__ == "__main__":
    rng = np.random.default_rng(0)
    x = rng.random((BS, 3, 512, 512), dtype=np.float32)
    y = kernel(x, 80)
    print("kernel ran:", y.shape, y.dtype, float(y.min()), float(y.max()))
